# revision 1
# baseline (speedup 1.0000x reference)
"""Bass/Tile TRN2 kernel for nn_BinaryTreeLSTM (B=256, L=32, D=512, H=512).

Strategy: pure data parallelism over batch (32 sequences per core, 8 cores).
Per-core algorithm mirrors the reference exactly:
  - leaf states via word_linear matmul
  - 31 serial shrink iterations; each composes all adjacent pairs,
    scores them with comp_query, argmaxes per sequence, and blends
    (select / keep-left / shift-right) with predicated copies.

Layout: states are feature-major [128part(feat%128), 4(feat//128), 32(b), n]
so the compose matmul (K=2H on partitions) needs no transposes.
"""

import sys
import numpy as np

sys.path.insert(0, "/opt/trn_rl_repo")
import ml_dtypes

from contextlib import ExitStack

import concourse.bass as bass
import concourse.tile as tile
from concourse import bacc, mybir
from concourse.bass_utils import run_bass_kernel_spmd

FP32 = mybir.dt.float32
BF16 = mybir.dt.bfloat16
U32 = mybir.dt.uint32
U8 = mybir.dt.uint8

B, L, D, H = 256, 32, 512, 512
NCORES = 8
BC = B // NCORES          # 32 sequences per core
NEG = -1.0e9
DEBUG = False
AF = mybir.ActivationFunctionType
OP = mybir.AluOpType


def build_kernel(ctx: ExitStack, tc: "tile.TileContext", io: dict):
    nc = tc.nc

    xT = io["xT"]          # [4, 128, BC*L]   x shard, feature-major
    wwT = io["wwT"]        # [4, 128, 2H]     W_word^T k-tiles
    wcT = io["wcT"]        # [8, 128, 5H]     W_comp^T k-tiles
    bw_d = io["bw"]        # [128, 8]
    bc_d = io["bc"]        # [128, 20]        (+1.0 folded into fl/fr)
    qv_d = io["qv"]        # [128, 4]         comp_query * scale
    lm_d = io["lm"]        # [BC, L]          length mask
    nlm_d = io["nlm"]      # [BC, L]          1 - length mask
    nlm8_d = io["nlm8"]    # [BC, L] uint8    1 - length mask
    iota_d = io["iota"]    # [BC, L-1]        iota[b, j] = j
    neg_d = io["neg"]      # [BC, L-1]        all NEG
    out_d = io["out"]      # [BC, 2H/2] = [32, 512]

    consts = ctx.enter_context(tc.tile_pool(name="consts", bufs=1))
    states = ctx.enter_context(tc.tile_pool(name="states", bufs=2))
    gates = ctx.enter_context(tc.tile_pool(name="gates", bufs=2))
    xpool = ctx.enter_context(tc.tile_pool(name="xpool", bufs=4))
    mpool = ctx.enter_context(tc.tile_pool(name="mpool", bufs=1))
    small = ctx.enter_context(tc.tile_pool(name="small", bufs=1))
    psum = ctx.enter_context(tc.tile_pool(name="psum", bufs=4 if DEBUG else 6, space="PSUM"))
    psuml = ctx.enter_context(tc.tile_pool(name="psuml", bufs=2, space="PSUM"))

    # ---- persistent constants in SBUF ----
    wc_sb = consts.tile([128, 8, 5 * H], FP32, tag="wc")
    for kt in range(8):
        nc.sync.dma_start(out=wc_sb[:, kt, :], in_=wcT[kt])
    ww_sb = mpool.tile([128, 4, 2 * H], FP32, tag="wwm")
    for kt in range(4):
        nc.sync.dma_start(out=ww_sb[:, kt, :], in_=wwT[kt])
    bw_sb = consts.tile([128, 8], FP32, tag="bw")
    nc.sync.dma_start(out=bw_sb[:], in_=bw_d[:])
    bc_sb = consts.tile([128, 20], FP32, tag="bc")
    nc.sync.dma_start(out=bc_sb[:], in_=bc_d[:])
    qv_sb = consts.tile([128, 4], FP32, tag="qv")
    nc.sync.dma_start(out=qv_sb[:], in_=qv_d[:])
    lm_sb = consts.tile([BC, L], FP32, tag="lm")
    nc.sync.dma_start(out=lm_sb[:], in_=lm_d[:])
    nlm_sb = consts.tile([BC, L], FP32, tag="nlm")
    nc.sync.dma_start(out=nlm_sb[:], in_=nlm_d[:])
    nlm8_sb = consts.tile([BC, L], U8, tag="nlm8")
    nc.sync.dma_start(out=nlm8_sb[:], in_=nlm8_d[:])
    iota_sb = consts.tile([BC, L - 1], FP32, tag="iota")
    nc.sync.dma_start(out=iota_sb[:], in_=iota_d[:])
    neg_sb = consts.tile([BC, L - 1], FP32, tag="neg")
    nc.sync.dma_start(out=neg_sb[:], in_=neg_d[:])

    # ---- leaf states: hc = W_word @ x  -> h (mt 0..3), c (mt 4..7) ----
    h_cur = states.tile([128, 4, BC, L], FP32, tag="h")
    c_cur = states.tile([128, 4, BC, L], FP32, tag="c")
    for cidx in range(2):        # halves of (b, n) free dim, 512 each
        xb = []
        for kt in range(4):
            xt = xpool.tile([128, 512], FP32, tag="xb")
            nc.sync.dma_start(out=xt[:], in_=xT[kt][:, cidx * 512:(cidx + 1) * 512])
            xb.append(xt)
        for m in range(8):
            ps = psum.tile([128, 512], FP32, tag="pg")
            for kt in range(4):
                nc.tensor.matmul(
                    ps[:], wc_lhsT(ww_sb, kt, m), xb[kt][:],
                    start=(kt == 0), stop=(kt == 3),
                )
            dst = h_cur if m < 4 else c_cur
            dview = dst[:, m % 4, 16 * cidx:16 * cidx + 16, :]
            nc.vector.tensor_scalar(
                dview, ps[:].rearrange("p (b n) -> p b n", b=16),
                bw_sb[:, m:m + 1], None, OP.add,
            )

    dbg = io.get("dbg")
    if dbg is not None:
        nc.sync.dma_start(out=dbg["hleaf"], in_=h_cur[:, 0, :, :])
        nc.sync.dma_start(out=dbg["cleaf"], in_=c_cur[:, 0, :, :])

    # ---- main loop ----
    for i in range(L - 1):
        p = L - 1 - i
        chunks = [(0, BC)] if BC * p <= 512 else [(0, 16), (16, BC)]

        h_nxt = states.tile([128, 4, BC, L], FP32, tag="h")
        c_nxt = states.tile([128, 4, BC, L], FP32, tag="c")

        pls = []
        for (b0, b1) in chunks:
            nb = b1 - b0
            N = nb * p

            def rhs(kt):
                if kt < 4:
                    return h_cur[:, kt, b0:b1, 0:p]
                return h_cur[:, kt - 4, b0:b1, 1:p + 1]

            for f in range(4):
                ps_g = []
                for g in range(5):
                    ps = psum.tile([128, 512], FP32, tag="pg")
                    mt = g * 4 + f
                    for kt in range(8):
                        nc.tensor.matmul(
                            ps[:, 0:N], wc_lhsT(wc_sb, kt, mt), rhs(kt),
                            start=(kt == 0), stop=(kt == 7),
                        )
                    ps_g.append(ps)

                def pview(t):
                    return t[:, 0:N].rearrange("p (b n) -> p b n", b=nb)

                cl = c_cur[:, f, b0:b1, 0:p]
                cr = c_cur[:, f, b0:b1, 1:p + 1]
                cn = c_nxt[:, f, b0:b1, 0:p]
                hn = h_nxt[:, f, b0:b1, 0:p]

                si = gates.tile([128, 512], FP32, tag="si")
                nc.scalar.activation(si[:, 0:N], ps_g[0][:, 0:N], AF.Sigmoid,
                                     bias=bc_sb[:, f:f + 1])
                t1 = gates.tile([128, 512], FP32, tag="tg")
                nc.scalar.activation(t1[:, 0:N], ps_g[1][:, 0:N], AF.Sigmoid,
                                     bias=bc_sb[:, 4 + f:5 + f])
                nc.vector.tensor_tensor(cn, pview(t1), cl, OP.mult)
                t2 = gates.tile([128, 512], FP32, tag="tg")
                nc.scalar.activation(t2[:, 0:N], ps_g[2][:, 0:N], AF.Sigmoid,
                                     bias=bc_sb[:, 8 + f:9 + f])
                nc.gpsimd.tensor_tensor(pview(t2), pview(t2), cr, OP.mult)
                nc.gpsimd.tensor_tensor(cn, cn, pview(t2), OP.add)
                t3 = gates.tile([128, 512], FP32, tag="tg")
                nc.scalar.activation(t3[:, 0:N], ps_g[3][:, 0:N], AF.Tanh,
                                     bias=bc_sb[:, 12 + f:13 + f])
                nc.gpsimd.tensor_tensor(pview(t3), pview(t3), pview(si), OP.mult)
                nc.gpsimd.tensor_tensor(cn, cn, pview(t3), OP.add)
                so = gates.tile([128, 512], FP32, tag="so")
                nc.scalar.activation(so[:, 0:N], ps_g[4][:, 0:N], AF.Sigmoid,
                                     bias=bc_sb[:, 16 + f:17 + f])
                tcn = gates.tile([128, 512], FP32, tag="tg")
                nc.scalar.activation(pview(tcn), cn, AF.Tanh)
                nc.vector.tensor_tensor(hn, pview(so), pview(tcn), OP.mult)

            # logits for this chunk: q . h_nxt
            if p == 1:
                continue
            pl = psuml.tile([1, 512], FP32, tag="pl")
            for f in range(4):
                nc.tensor.matmul(
                    pl[:, 0:N], qv_sb[:, f:f + 1], h_nxt[:, f, b0:b1, 0:p],
                    start=(f == 0), stop=(f == 3),
                )
            pls.append((pl, b0, b1, N))
            if dbg is not None and i == 0:
                pr = psuml.tile([1, 512], FP32, tag="plr")
                for f in range(4):
                    nc.tensor.matmul(
                        pr[:, 0:N], qv_sb[:, f:f + 1], h_nxt[:, f, b0:b1, 0:p],
                        start=(f == 0), stop=(f == 3),
                    )
                rrow = small.tile([1, 512], FP32, tag="rrow")
                nc.scalar.copy(rrow[:, 0:N], pr[:, 0:N])
                nc.sync.dma_start(out=dbg[f"lrec_{b0}"], in_=rrow[:, 0:N])

        # ---- selection masks S3 = [sel*done | left*done + (1-done) | right*done]
        S3 = small.tile([BC, 3 * (L - 1)], U8, tag="s3")
        if p > 1:
            lfull = small.tile([BC, L - 1], FP32, tag="lf")
            nc.vector.tensor_copy(lfull[:], neg_sb[:])
            for (pl, b0, b1, N) in pls:
                nb = b1 - b0
                lrow = small.tile([1, 512], FP32, tag="lrow")
                nc.scalar.copy(lrow[:, 0:N], pl[:, 0:N])
                if dbg is not None and i == 0:
                    nc.sync.dma_start(out=dbg[f"lrow_{b0}"], in_=lrow[:, 0:N])
                nc.sync.dma_start(out=lfull[b0:b1, 0:p], in_=lrow[:, 0:N])
            nc.vector.copy_predicated(
                lfull[:, 0:p], nlm8_sb[:, i + 1:i + 1 + p], neg_sb[:, 0:p])
            mx = small.tile([BC, 8], FP32, tag="mx")
            nc.vector.max(mx[:], lfull[:])
            ix = small.tile([BC, 8], U32, tag="ix")
            nc.vector.max_index(ix[:], mx[:], lfull[:])
            ixf = small.tile([BC, 1], FP32, tag="ixf")
            nc.vector.tensor_copy(ixf[:], ix[:, 0:1])

            done = lm_sb[:, i + 1:i + 2]
            ndone = nlm_sb[:, i + 1:i + 2]
            nc.vector.tensor_scalar(
                S3[:, 0:L - 1], iota_sb[:], ixf[:], done, OP.is_equal, OP.mult)
            tl = small.tile([BC, L - 1], FP32, tag="tl")
            nc.vector.tensor_scalar(
                tl[:], iota_sb[:], ixf[:], done, OP.is_lt, OP.mult)
            nc.vector.tensor_scalar(
                S3[:, L - 1:2 * (L - 1)], tl[:], ndone[:], None, OP.add)
            nc.vector.tensor_scalar(
                S3[:, 2 * (L - 1):], iota_sb[:], ixf[:], done, OP.is_gt, OP.mult)
        else:
            # last merge: no selection; sel = col 0, done-blend only
            nc.vector.memset(S3[:], 0.0)
            nc.vector.tensor_copy(S3[:, 0:1], lm_sb[:, L - 1:L])
            nc.vector.tensor_copy(S3[:, L - 1:L], nlm_sb[:, L - 1:L])

        if dbg is not None and i == 0:
            nc.sync.dma_start(out=dbg["lfull0"], in_=lfull[:])
            nc.sync.dma_start(out=dbg["ix0"], in_=ix[:])
            nc.sync.dma_start(out=dbg["s30"], in_=S3[:])
            nc.sync.dma_start(out=dbg["hcand0"], in_=h_nxt[:, 0, :, 0:31])
            nc.sync.dma_start(out=dbg["ccand0"], in_=c_nxt[:, 0, :, 0:31])
        if dbg is not None and i == 1:
            nc.sync.dma_start(out=dbg["lfull1"], in_=lfull[:])
            nc.sync.dma_start(out=dbg["ix1"], in_=ix[:])
        s3row = small.tile([1, BC * 3 * (L - 1)], U8, tag="s3r")
        S3v = S3[:].rearrange("b (m k) -> b m k", m=3)
        nc.sync.dma_start(out=s3row[:, 0:BC * 3 * p], in_=S3v[:, :, 0:p])
        M128 = mpool.tile([128, BC * 3 * (L - 1)], U8, tag="wwm")
        nc.gpsimd.partition_broadcast(M128[:, 0:BC * 3 * p], s3row[:, 0:BC * 3 * p])
        Mv = M128[:, 0:BC * 3 * p].rearrange("q (b m k) -> q b m k", b=BC, m=3)

        def bmask(m):
            return Mv[:, :, m, :]

        for f in range(4):
            nc.vector.copy_predicated(
                h_nxt[:, f, :, 0:p], bmask(1), h_cur[:, f, :, 0:p])
            nc.vector.copy_predicated(
                h_nxt[:, f, :, 0:p], bmask(2), h_cur[:, f, :, 1:p + 1])
            nc.vector.copy_predicated(
                c_nxt[:, f, :, 0:p], bmask(1), c_cur[:, f, :, 0:p])
            nc.vector.copy_predicated(
                c_nxt[:, f, :, 0:p], bmask(2), c_cur[:, f, :, 1:p + 1])

        if dbg is not None and i == 0:
            nc.sync.dma_start(out=dbg["m0"], in_=M128[:, 0:93])
            nc.sync.dma_start(out=dbg["hpost0"], in_=h_nxt[:, 0, :, 0:31])
        h_cur, c_cur = h_nxt, c_nxt

    # ---- output: h root, feature-major -> [b, 512] ----
    ocp = consts.tile([128, 4, BC], FP32, tag="ocp")
    nc.vector.tensor_copy(ocp[:], h_cur[:, :, :, 0])
    out_v = out_d.rearrange("b (f q) -> f q b", f=4)
    for f in range(4):
        nc.sync.dma_start(out=out_v[f], in_=ocp[:, f, :])


def wc_lhsT(w_sb, kt, mt):
    return w_sb[:, kt, mt * 128:(mt + 1) * 128]


_BUILD_CACHE = {}


def build():
    if "nc" in _BUILD_CACHE:
        return _BUILD_CACHE["nc"]
    nc = bacc.Bacc("TRN2", target_bir_lowering=False, debug=False)
    io = {
        "xT": nc.dram_tensor("xT", [4, 128, BC * L], FP32, kind="ExternalInput").ap(),
        "wwT": nc.dram_tensor("wwT", [4, 128, 2 * H], FP32, kind="ExternalInput").ap(),
        "wcT": nc.dram_tensor("wcT", [8, 128, 5 * H], FP32, kind="ExternalInput").ap(),
        "bw": nc.dram_tensor("bw", [128, 8], FP32, kind="ExternalInput").ap(),
        "bc": nc.dram_tensor("bc", [128, 20], FP32, kind="ExternalInput").ap(),
        "qv": nc.dram_tensor("qv", [128, 4], FP32, kind="ExternalInput").ap(),
        "lm": nc.dram_tensor("lm", [BC, L], FP32, kind="ExternalInput").ap(),
        "nlm": nc.dram_tensor("nlm", [BC, L], FP32, kind="ExternalInput").ap(),
        "nlm8": nc.dram_tensor("nlm8", [BC, L], U8, kind="ExternalInput").ap(),
        "iota": nc.dram_tensor("iota", [BC, L - 1], FP32, kind="ExternalInput").ap(),
        "neg": nc.dram_tensor("neg", [BC, L - 1], FP32, kind="ExternalInput").ap(),
        "out": nc.dram_tensor("out", [BC, H], FP32, kind="ExternalOutput").ap(),
    }
    if DEBUG:
        io["dbg"] = {
            "hleaf": nc.dram_tensor("hleaf", [128, BC, L], FP32, kind="ExternalOutput").ap(),
            "cleaf": nc.dram_tensor("cleaf", [128, BC, L], FP32, kind="ExternalOutput").ap(),
            "lfull0": nc.dram_tensor("lfull0", [BC, L - 1], FP32, kind="ExternalOutput").ap(),
            "ix0": nc.dram_tensor("ix0", [BC, 8], U32, kind="ExternalOutput").ap(),
            "s30": nc.dram_tensor("s30", [BC, 3 * (L - 1)], U8, kind="ExternalOutput").ap(),
            "hcand0": nc.dram_tensor("hcand0", [128, BC, L - 1], FP32, kind="ExternalOutput").ap(),
            "ccand0": nc.dram_tensor("ccand0", [128, BC, L - 1], FP32, kind="ExternalOutput").ap(),
            "m0": nc.dram_tensor("m0", [128, 3 * (L - 1)], U8, kind="ExternalOutput").ap(),
            "hpost0": nc.dram_tensor("hpost0", [128, BC, L - 1], FP32, kind="ExternalOutput").ap(),
            "lfull1": nc.dram_tensor("lfull1", [BC, L - 1], FP32, kind="ExternalOutput").ap(),
            "ix1": nc.dram_tensor("ix1", [BC, 8], U32, kind="ExternalOutput").ap(),
            "lrow_0": nc.dram_tensor("lrow_0", [1, 496], FP32, kind="ExternalOutput").ap(),
            "lrow_16": nc.dram_tensor("lrow_16", [1, 496], FP32, kind="ExternalOutput").ap(),
            "lrec_0": nc.dram_tensor("lrec_0", [1, 496], FP32, kind="ExternalOutput").ap(),
            "lrec_16": nc.dram_tensor("lrec_16", [1, 496], FP32, kind="ExternalOutput").ap(),
        }
    with tile.TileContext(nc) as tc:
        with ExitStack() as ctx:
            build_kernel(ctx, tc, io)
    nc.compile()
    _BUILD_CACHE["nc"] = nc
    return nc


def make_in_maps(x, length, W_word, b_word, W_comp, b_comp, comp_query):
    x = np.asarray(x, np.float32)
    length = np.asarray(length)
    W_word = np.asarray(W_word, np.float32)
    b_word = np.asarray(b_word, np.float32)
    W_comp = np.asarray(W_comp, np.float32)
    b_comp = np.asarray(b_comp, np.float32)
    comp_query = np.asarray(comp_query, np.float32)

    wwT = np.ascontiguousarray(W_word.T.reshape(4, 128, 2 * H))
    wcT = np.ascontiguousarray(W_comp.T.reshape(8, 128, 5 * H))
    bw = np.ascontiguousarray(b_word.reshape(8, 128).T)
    bca = b_comp.copy()
    bca[H:3 * H] += 1.0     # forget-gate biases fl, fr
    bc = np.ascontiguousarray(bca.reshape(20, 128).T)
    qv = np.ascontiguousarray(
        (comp_query * (1.0 / np.sqrt(H))).astype(np.float32).reshape(4, 128).T)
    lm_full = (np.arange(L)[None, :] < length[:, None]).astype(np.float32)
    iota = np.tile(np.arange(L - 1, dtype=np.float32), (BC, 1))
    negt = np.full((BC, L - 1), NEG, np.float32)

    in_maps = []
    for k in range(NCORES):
        xs = x[k * BC:(k + 1) * BC]                       # [BC, L, D]
        xT = np.ascontiguousarray(
            xs.transpose(2, 0, 1).reshape(4, 128, BC * L))
        lm = np.ascontiguousarray(lm_full[k * BC:(k + 1) * BC])
        in_maps.append({
            "xT": xT, "wwT": wwT, "wcT": wcT, "bw": bw, "bc": bc,
            "qv": qv, "lm": lm, "nlm": np.ascontiguousarray(1.0 - lm),
            "nlm8": np.ascontiguousarray((1.0 - lm).astype(np.uint8)),
            "iota": iota, "neg": negt,
        })
    return in_maps


def kernel(x, length, W_word, b_word, W_comp, b_comp, comp_query):
    nc = build()
    in_maps = make_in_maps(x, length, W_word, b_word, W_comp, b_comp, comp_query)
    res = run_bass_kernel_spmd(nc, in_maps, list(range(NCORES)))
    return np.concatenate([res.results[k]["out"] for k in range(NCORES)], axis=0)



# revision 25
# speedup vs baseline: 5.7322x; 5.7322x over previous
"""Bass/Tile TRN2 kernel for nn_BinaryTreeLSTM (B=256, L=32, D=512, H=512).

Strategy: pure data parallelism over batch (32 sequences per core, 8 cores).
Per-core algorithm mirrors the reference exactly:
  - leaf states via word_linear matmul
  - 31 serial shrink iterations; each composes all adjacent pairs,
    scores them with comp_query, argmaxes per sequence, and blends
    (select / keep-left / shift-right) with predicated copies.

Layout: states are feature-major [128part(feat%128), 4(feat//128), 32(b), n]
so the compose matmul (K=2H on partitions) needs no transposes.
"""

import sys
import numpy as np

sys.path.insert(0, "/opt/trn_rl_repo")
import ml_dtypes

from contextlib import ExitStack

import concourse.bass as bass
import concourse.tile as tile
from concourse import bacc, mybir
from concourse.bass_utils import run_bass_kernel_spmd

FP32 = mybir.dt.float32
FP32R = mybir.dt.float32r
BF16 = mybir.dt.bfloat16
U32 = mybir.dt.uint32
U8 = mybir.dt.uint8


def r32(ap):
    """View an fp32 AP as float32r: same bits, 4x faster PE when N>=256."""
    return ap.bitcast(FP32R)


def round12(a):
    """Round fp32 array to fp32r precision (12 mantissa bits incl implicit,
    round-to-nearest-even). Bit-identical to neuronxcc static_cast fp32->fp32r."""
    a = np.ascontiguousarray(a, np.float32)
    ai = a.view(np.uint32).copy()
    shift = 12
    lsb = np.uint32(1) << np.uint32(shift)
    half = lsb >> np.uint32(1)
    rem = ai & (lsb - np.uint32(1))
    ai &= ~(lsb - np.uint32(1))
    up = (rem > half) | ((rem == half) & ((ai & lsb) != 0))
    ai += np.where(up, lsb, np.uint32(0))
    return ai.view(np.float32)


def even2(x):
    return x + (x & 1)

B, L, D, H = 256, 32, 512, 512
NCORES = 8
BC = B // NCORES          # 32 sequences per core
NEG = -1.0e9
DEBUG = False
AF = mybir.ActivationFunctionType
OP = mybir.AluOpType


def build_kernel(ctx: ExitStack, tc: "tile.TileContext", io: dict):
    nc = tc.nc

    xT = io["xT"]          # [4, 128, BC*L]   x shard, feature-major
    wwT = io["wwT"]        # [4, 128, 2H]     W_word^T k-tiles
    wcT = io["wcT"]        # [8, 128, 5H]     W_comp^T k-tiles
    bw_d = io["bw"]        # [128, 8]
    bc_d = io["bc"]        # [128, 20]        (+1.0 folded into fl/fr)
    qv_d = io["qv"]        # [128, 4]         comp_query * scale
    lm_d = io["lm"]        # [BC, L]          length mask
    nlm_d = io["nlm"]      # [BC, L]          1 - length mask
    nlm8_d = io["nlm8"]    # [BC, L] uint8    1 - length mask
    iota_d = io["iota"]    # [BC, L-1]        iota[b, j] = j
    neg_d = io["neg"]      # [BC, L-1]        all NEG
    out_d = io["out"]      # [BC, 2H/2] = [32, 512]

    consts = ctx.enter_context(tc.tile_pool(name="consts", bufs=1))
    states = ctx.enter_context(tc.tile_pool(name="states", bufs=2))
    gates = ctx.enter_context(tc.tile_pool(name="gates", bufs=2))
    xpool = ctx.enter_context(tc.tile_pool(name="xpool", bufs=4))
    mpool = ctx.enter_context(tc.tile_pool(name="mpool", bufs=1))
    small = ctx.enter_context(tc.tile_pool(name="small", bufs=1))
    psum = ctx.enter_context(tc.tile_pool(name="psum", bufs=4 if DEBUG else 6, space="PSUM"))
    psuml = ctx.enter_context(tc.tile_pool(name="psuml", bufs=2, space="PSUM"))

    # ---- persistent constants in SBUF ----
    wc_sb = consts.tile([128, 8, 5 * H], FP32R, tag="wc")
    for kt in range(8):
        nc.sync.dma_start(out=wc_sb[:, kt, :], in_=wcT[kt])
    ww_sb = mpool.tile([128, 4, 2 * H], FP32R, tag="wwm")
    for kt in range(4):
        nc.sync.dma_start(out=ww_sb[:, kt, :], in_=wwT[kt])
    bw_sb = consts.tile([128, 8], FP32, tag="bw")
    nc.sync.dma_start(out=bw_sb[:], in_=bw_d[:])
    bc_sb = consts.tile([128, 20], FP32, tag="bc")
    nc.sync.dma_start(out=bc_sb[:], in_=bc_d[:])
    qv_sb = consts.tile([128, 4], FP32, tag="qv")
    nc.sync.dma_start(out=qv_sb[:], in_=qv_d[:])
    z1 = consts.tile([128, 1], FP32, tag="z1")
    nc.vector.memset(z1[:], 0.0)
    lm_sb = consts.tile([BC, L], FP32, tag="lm")
    nc.sync.dma_start(out=lm_sb[:], in_=lm_d[:])
    nlm_sb = consts.tile([BC, L], FP32, tag="nlm")
    nc.sync.dma_start(out=nlm_sb[:], in_=nlm_d[:])
    nlm8_sb = consts.tile([BC, L], U8, tag="nlm8")
    nc.sync.dma_start(out=nlm8_sb[:], in_=nlm8_d[:])
    iota_sb = consts.tile([BC, L - 1], FP32, tag="iota")
    nc.sync.dma_start(out=iota_sb[:], in_=iota_d[:])
    neg_sb = consts.tile([BC, L - 1], FP32, tag="neg")
    nc.sync.dma_start(out=neg_sb[:], in_=neg_d[:])

    # ---- leaf states: hc = W_word @ x  -> h (mt 0..3), c (mt 4..7) ----
    LW = L + 2   # state tiles 2 cols wider: cover-even-width reads for fp32r
    h_cur = states.tile([128, 4, BC, LW], FP32, tag="h")
    c_cur = states.tile([128, 4, BC, LW], FP32, tag="c")
    for cidx in range(2):        # halves of (b, n) free dim, 512 each
        xb = []
        for kt in range(4):
            xt = xpool.tile([128, 512], FP32R, tag="xb")
            nc.sync.dma_start(out=xt[:], in_=xT[kt][:, cidx * 512:(cidx + 1) * 512])
            xb.append(xt)
        for m in range(8):
            ps = psum.tile([128, 512], FP32, tag="pg")
            for kt in range(4):
                nc.tensor.matmul(
                    ps[:], wc_lhsT(ww_sb, kt, m), xb[kt][:],
                    start=(kt == 0), stop=(kt == 3),
                )
            dst = h_cur if m < 4 else c_cur
            dview = dst[:, m % 4, 16 * cidx:16 * cidx + 16, 0:L]
            nc.vector.tensor_scalar(
                dview, ps[:].rearrange("p (b n) -> p b n", b=16),
                bw_sb[:, m:m + 1], None, OP.add,
            )
    nc.vector.memset(c_cur[:, :, :, L:LW], 0.0)
    nc.vector.memset(h_cur[:, :, :, L:LW], 0.0)
    # rounded copy of h for fp32r compose matmuls (tensor_scalar +0 rounds)
    h_r = states.tile([128, 4, BC, LW], FP32R, tag="hr", bufs=1)
    for f4 in range(4):
        eng = nc.vector if f4 % 2 == 0 else nc.gpsimd
        eng.tensor_scalar(h_r[:, f4, :, :], h_cur[:, f4, :, :],
                          z1[:], None, OP.add)

    dbg = io.get("dbg")
    if dbg is not None:
        nc.sync.dma_start(out=dbg["hleaf"], in_=h_cur[:, 0, :, :])
        nc.sync.dma_start(out=dbg["cleaf"], in_=c_cur[:, 0, :, :])

    # ---- main loop ----
    for i in range(L - 1):
        p = L - 1 - i
        chunks = [(0, BC)] if BC * p <= 512 else [(0, 16), (16, BC)]

        h_nxt = states.tile([128, 4, BC, LW], FP32, tag="h")
        c_nxt = states.tile([128, 4, BC, LW], FP32, tag="c")

        p_e = even2(max(p, 8))   # compose width: even (fp32r) and >=8 (N>=256)
        pls = []
        for (b0, b1) in chunks:
            nb = b1 - b0
            N = nb * p
            NE = nb * p_e

            def rhs(kt):
                if kt < 4:
                    return h_r[:, kt, b0:b1, 0:p_e]
                return h_r[:, kt - 4, b0:b1, 1:p_e + 1]

            for f in range(4):
                ps_g = []
                for g in range(5):
                    ps = psum.tile([128, 512], FP32, tag="pg")
                    mt = g * 4 + f
                    for kt in range(8):
                        nc.tensor.matmul(
                            ps[:, 0:NE], wc_lhsT(wc_sb, kt, mt), rhs(kt),
                            start=(kt == 0), stop=(kt == 7),
                        )
                    ps_g.append(ps)

                def pview(t):
                    return t[:, 0:NE].rearrange("p (b n) -> p b n", b=nb)

                cl = c_cur[:, f, b0:b1, 0:p_e]
                cr = c_cur[:, f, b0:b1, 1:p_e + 1]
                cn = c_nxt[:, f, b0:b1, 0:p_e]
                hn = h_nxt[:, f, b0:b1, 0:p_e]

                si = gates.tile([128, 512], FP32, tag="si")
                nc.scalar.activation(si[:, 0:NE], ps_g[0][:, 0:NE], AF.Sigmoid,
                                     bias=bc_sb[:, f:f + 1])
                t1 = gates.tile([128, 512], FP32, tag="tg")
                nc.scalar.activation(t1[:, 0:NE], ps_g[1][:, 0:NE], AF.Sigmoid,
                                     bias=bc_sb[:, 4 + f:5 + f])
                nc.vector.tensor_tensor(cn, pview(t1), cl, OP.mult)
                t2 = gates.tile([128, 512], FP32, tag="tg")
                nc.scalar.activation(t2[:, 0:NE], ps_g[2][:, 0:NE], AF.Sigmoid,
                                     bias=bc_sb[:, 8 + f:9 + f])
                nc.gpsimd.tensor_tensor(pview(t2), pview(t2), cr, OP.mult)
                nc.gpsimd.tensor_tensor(cn, cn, pview(t2), OP.add)
                t3 = gates.tile([128, 512], FP32, tag="tg")
                nc.scalar.activation(t3[:, 0:NE], ps_g[3][:, 0:NE], AF.Tanh,
                                     bias=bc_sb[:, 12 + f:13 + f])
                nc.gpsimd.tensor_tensor(pview(t3), pview(t3), pview(si), OP.mult)
                nc.gpsimd.tensor_tensor(cn, cn, pview(t3), OP.add)
                so = gates.tile([128, 512], FP32, tag="so")
                nc.scalar.activation(so[:, 0:NE], ps_g[4][:, 0:NE], AF.Sigmoid,
                                     bias=bc_sb[:, 16 + f:17 + f])
                tcn = gates.tile([128, 512], FP32, tag="tg")
                nc.scalar.activation(pview(tcn), cn, AF.Tanh)
                nc.vector.tensor_tensor(hn, pview(so), pview(tcn), OP.mult)

            # logits for this chunk: q . h_nxt  (even width for fp32r rules)
            if p == 1:
                continue
            wl = even2(p)
            pl = psuml.tile([1, 512], FP32, tag="pl")
            for f in range(4):
                nc.tensor.matmul(
                    pl[:, 0:nb * wl], qv_sb[:, f:f + 1],
                    h_nxt[:, f, b0:b1, 0:wl],
                    start=(f == 0), stop=(f == 3),
                )
            pls.append((pl, b0, b1, nb * wl, wl))
            if dbg is not None and i == 0:
                pr = psuml.tile([1, 512], FP32, tag="plr")
                for f in range(4):
                    nc.tensor.matmul(
                        pr[:, 0:N], qv_sb[:, f:f + 1], h_nxt[:, f, b0:b1, 0:p],
                        start=(f == 0), stop=(f == 3),
                    )
                rrow = small.tile([1, 512], FP32, tag="rrow")
                nc.scalar.copy(rrow[:, 0:N], pr[:, 0:N])
                nc.sync.dma_start(out=dbg[f"lrec_{b0}"], in_=rrow[:, 0:N])

        if p > 1:
            # cover the two cols past the gates' write range so next
            # iteration's even-width fp32r reads stay inside written memory
            nc.vector.memset(c_nxt[:, :, :, p_e:p_e + 2], 0.0)
            nc.vector.memset(h_nxt[:, :, :, p_e:p_e + 2], 0.0)

        # ---- selection masks S3 = [sel*done | left*done + (1-done) | right*done]
        S3 = small.tile([BC, 3 * (L - 1)], U8, tag="s3")
        if p > 1:
            lfull = small.tile([BC, L], FP32, tag="lf")
            nc.vector.tensor_copy(lfull[:, 0:L - 1], neg_sb[:])
            for (pl, b0, b1, N_l, wl) in pls:
                nb = b1 - b0
                lrow = small.tile([1, 512], FP32, tag="lrow")
                nc.scalar.copy(lrow[:, 0:N_l], pl[:, 0:N_l])
                if dbg is not None and i == 0:
                    nc.sync.dma_start(out=dbg[f"lrow_{b0}"], in_=lrow[:, 0:N_l])
                nc.sync.dma_start(out=lfull[b0:b1, 0:wl], in_=lrow[:, 0:N_l])
            if even2(p) > p and p < L - 1:
                # junk col from even-width logits matmul: force back to NEG
                nc.vector.tensor_copy(lfull[:, p:p + 1], neg_sb[:, p:p + 1])
            nc.vector.copy_predicated(
                lfull[:, 0:p], nlm8_sb[:, i + 1:i + 1 + p], neg_sb[:, 0:p])
            mx = small.tile([BC, 8], FP32, tag="mx")
            nc.vector.max(mx[:], lfull[:, 0:L - 1])
            ix = small.tile([BC, 8], U32, tag="ix")
            nc.vector.max_index(ix[:], mx[:], lfull[:, 0:L - 1])
            ixf = small.tile([BC, 1], FP32, tag="ixf")
            nc.vector.tensor_copy(ixf[:], ix[:, 0:1])

            done = lm_sb[:, i + 1:i + 2]
            ndone = nlm_sb[:, i + 1:i + 2]
            nc.vector.tensor_scalar(
                S3[:, 0:L - 1], iota_sb[:], ixf[:], done, OP.is_equal, OP.mult)
            tl = small.tile([BC, L - 1], FP32, tag="tl")
            nc.vector.tensor_scalar(
                tl[:], iota_sb[:], ixf[:], done, OP.is_lt, OP.mult)
            nc.vector.tensor_scalar(
                S3[:, L - 1:2 * (L - 1)], tl[:], ndone[:], None, OP.add)
            nc.vector.tensor_scalar(
                S3[:, 2 * (L - 1):], iota_sb[:], ixf[:], done, OP.is_gt, OP.mult)
        else:
            # last merge: no selection; sel = col 0, done-blend only
            nc.vector.memset(S3[:], 0.0)
            nc.vector.tensor_copy(S3[:, 0:1], lm_sb[:, L - 1:L])
            nc.vector.tensor_copy(S3[:, L - 1:L], nlm_sb[:, L - 1:L])

        if dbg is not None and i == 0:
            nc.sync.dma_start(out=dbg["lfull0"], in_=lfull[:])
            nc.sync.dma_start(out=dbg["ix0"], in_=ix[:])
            nc.sync.dma_start(out=dbg["s30"], in_=S3[:])
            nc.sync.dma_start(out=dbg["hcand0"], in_=h_nxt[:, 0, :, 0:31])
            nc.sync.dma_start(out=dbg["ccand0"], in_=c_nxt[:, 0, :, 0:31])
        if dbg is not None and i == 1:
            nc.sync.dma_start(out=dbg["lfull1"], in_=lfull[:])
            nc.sync.dma_start(out=dbg["ix1"], in_=ix[:])
        s3row = small.tile([1, BC * 3 * (L - 1)], U8, tag="s3r")
        S3v = S3[:].rearrange("b (m k) -> b m k", m=3)
        nc.sync.dma_start(out=s3row[:, 0:BC * 3 * p], in_=S3v[:, :, 0:p])
        M128 = mpool.tile([128, BC * 3 * (L - 1)], U8, tag="wwm")
        nc.gpsimd.partition_broadcast(M128[:, 0:BC * 3 * p], s3row[:, 0:BC * 3 * p])
        Mv = M128[:, 0:BC * 3 * p].rearrange("q (b m k) -> q b m k", b=BC, m=3)

        def bmask(m):
            return Mv[:, :, m, :]

        for f in range(4):
            nc.vector.copy_predicated(
                h_nxt[:, f, :, 0:p], bmask(1), h_cur[:, f, :, 0:p])
            nc.vector.copy_predicated(
                h_nxt[:, f, :, 0:p], bmask(2), h_cur[:, f, :, 1:p + 1])
            nc.vector.copy_predicated(
                c_nxt[:, f, :, 0:p], bmask(1), c_cur[:, f, :, 0:p])
            nc.vector.copy_predicated(
                c_nxt[:, f, :, 0:p], bmask(2), c_cur[:, f, :, 1:p + 1])

        if dbg is not None and i == 0:
            nc.sync.dma_start(out=dbg["m0"], in_=M128[:, 0:93])
            nc.sync.dma_start(out=dbg["hpost0"], in_=h_nxt[:, 0, :, 0:31])
        if p > 1:
            pe_n = even2(max(p - 1, 8))   # next iteration's read width
            h_r = states.tile([128, 4, BC, LW], FP32R, tag="hr", bufs=1)
            for f4 in range(4):
                eng = nc.vector if f4 % 2 == 0 else nc.gpsimd
                eng.tensor_scalar(h_r[:, f4, :, 0:pe_n + 2],
                                  h_nxt[:, f4, :, 0:pe_n + 2],
                                  z1[:], None, OP.add)
        h_cur, c_cur = h_nxt, c_nxt

    # ---- output: h root, feature-major -> [b, 512] ----
    ocp = consts.tile([128, 4, BC], FP32, tag="ocp")
    nc.vector.tensor_copy(ocp[:], h_cur[:, :, :, 0])
    out_v = out_d.rearrange("b (f q) -> f q b", f=4)
    for f in range(4):
        nc.sync.dma_start(out=out_v[f], in_=ocp[:, f, :])


def wc_lhsT(w_sb, kt, mt):
    return w_sb[:, kt, mt * 128:(mt + 1) * 128]


_BUILD_CACHE = {}


def build():
    if "nc" in _BUILD_CACHE:
        return _BUILD_CACHE["nc"]
    nc = bacc.Bacc("TRN2", target_bir_lowering=False, debug=False)
    io = {
        "xT": nc.dram_tensor("xT", [4, 128, BC * L], FP32R, kind="ExternalInput").ap(),
        "wwT": nc.dram_tensor("wwT", [4, 128, 2 * H], FP32R, kind="ExternalInput").ap(),
        "wcT": nc.dram_tensor("wcT", [8, 128, 5 * H], FP32R, kind="ExternalInput").ap(),
        "bw": nc.dram_tensor("bw", [128, 8], FP32, kind="ExternalInput").ap(),
        "bc": nc.dram_tensor("bc", [128, 20], FP32, kind="ExternalInput").ap(),
        "qv": nc.dram_tensor("qv", [128, 4], FP32, kind="ExternalInput").ap(),
        "lm": nc.dram_tensor("lm", [BC, L], FP32, kind="ExternalInput").ap(),
        "nlm": nc.dram_tensor("nlm", [BC, L], FP32, kind="ExternalInput").ap(),
        "nlm8": nc.dram_tensor("nlm8", [BC, L], U8, kind="ExternalInput").ap(),
        "iota": nc.dram_tensor("iota", [BC, L - 1], FP32, kind="ExternalInput").ap(),
        "neg": nc.dram_tensor("neg", [BC, L - 1], FP32, kind="ExternalInput").ap(),
        "out": nc.dram_tensor("out", [BC, H], FP32, kind="ExternalOutput").ap(),
    }
    if DEBUG:
        io["dbg"] = {
            "hleaf": nc.dram_tensor("hleaf", [128, BC, L], FP32, kind="ExternalOutput").ap(),
            "cleaf": nc.dram_tensor("cleaf", [128, BC, L], FP32, kind="ExternalOutput").ap(),
            "lfull0": nc.dram_tensor("lfull0", [BC, L - 1], FP32, kind="ExternalOutput").ap(),
            "ix0": nc.dram_tensor("ix0", [BC, 8], U32, kind="ExternalOutput").ap(),
            "s30": nc.dram_tensor("s30", [BC, 3 * (L - 1)], U8, kind="ExternalOutput").ap(),
            "hcand0": nc.dram_tensor("hcand0", [128, BC, L - 1], FP32, kind="ExternalOutput").ap(),
            "ccand0": nc.dram_tensor("ccand0", [128, BC, L - 1], FP32, kind="ExternalOutput").ap(),
            "m0": nc.dram_tensor("m0", [128, 3 * (L - 1)], U8, kind="ExternalOutput").ap(),
            "hpost0": nc.dram_tensor("hpost0", [128, BC, L - 1], FP32, kind="ExternalOutput").ap(),
            "lfull1": nc.dram_tensor("lfull1", [BC, L - 1], FP32, kind="ExternalOutput").ap(),
            "ix1": nc.dram_tensor("ix1", [BC, 8], U32, kind="ExternalOutput").ap(),
            "lrow_0": nc.dram_tensor("lrow_0", [1, 496], FP32, kind="ExternalOutput").ap(),
            "lrow_16": nc.dram_tensor("lrow_16", [1, 496], FP32, kind="ExternalOutput").ap(),
            "lrec_0": nc.dram_tensor("lrec_0", [1, 496], FP32, kind="ExternalOutput").ap(),
            "lrec_16": nc.dram_tensor("lrec_16", [1, 496], FP32, kind="ExternalOutput").ap(),
        }
    with tile.TileContext(nc) as tc:
        with ExitStack() as ctx:
            build_kernel(ctx, tc, io)
    nc.compile()
    _BUILD_CACHE["nc"] = nc
    return nc


def make_in_maps(x, length, W_word, b_word, W_comp, b_comp, comp_query):
    x = np.asarray(x, np.float32)
    length = np.asarray(length)
    W_word = np.asarray(W_word, np.float32)
    b_word = np.asarray(b_word, np.float32)
    W_comp = np.asarray(W_comp, np.float32)
    b_comp = np.asarray(b_comp, np.float32)
    comp_query = np.asarray(comp_query, np.float32)

    wwT = round12(np.ascontiguousarray(W_word.T.reshape(4, 128, 2 * H)))
    wcT = round12(np.ascontiguousarray(W_comp.T.reshape(8, 128, 5 * H)))
    bw = np.ascontiguousarray(b_word.reshape(8, 128).T)
    bca = b_comp.copy()
    bca[H:3 * H] += 1.0     # forget-gate biases fl, fr
    bc = np.ascontiguousarray(bca.reshape(20, 128).T)
    qv = np.ascontiguousarray(
        (comp_query * (1.0 / np.sqrt(H))).astype(np.float32).reshape(4, 128).T)
    lm_full = (np.arange(L)[None, :] < length[:, None]).astype(np.float32)
    iota = np.tile(np.arange(L - 1, dtype=np.float32), (BC, 1))
    negt = np.full((BC, L - 1), NEG, np.float32)

    in_maps = []
    for k in range(NCORES):
        xs = x[k * BC:(k + 1) * BC]                       # [BC, L, D]
        xT = round12(np.ascontiguousarray(
            xs.transpose(2, 0, 1).reshape(4, 128, BC * L)))
        lm = np.ascontiguousarray(lm_full[k * BC:(k + 1) * BC])
        in_maps.append({
            "xT": xT, "wwT": wwT, "wcT": wcT, "bw": bw, "bc": bc,
            "qv": qv, "lm": lm, "nlm": np.ascontiguousarray(1.0 - lm),
            "nlm8": np.ascontiguousarray((1.0 - lm).astype(np.uint8)),
            "iota": iota, "neg": negt,
        })
    return in_maps


def kernel(x, length, W_word, b_word, W_comp, b_comp, comp_query):
    nc = build()
    in_maps = make_in_maps(x, length, W_word, b_word, W_comp, b_comp, comp_query)
    res = run_bass_kernel_spmd(nc, in_maps, list(range(NCORES)))
    return np.concatenate([res.results[k]["out"] for k in range(NCORES)], axis=0)

